# revision 12
# baseline (speedup 1.0000x reference)
"""AffinityLoss (segment-reduce) Trainium2 kernel.

Math (single pass over the data — no per-row center gather needed):
    lbl     = argmax(labels, axis=1)                         (N,)
    sums_c  = sum of features rows with lbl == c             (C, D)
    n_c     = count of rows with lbl == c                    (C,)
    sumsq   = sum(features ** 2)                             scalar
    centers = where(n>0, sums/max(n,1), 0) + 1e-6
    intra   = sumsq - 2*sum(sums*centers) + sum(n_c*||c_c||^2)
    inter   = sum((centers - mean(centers))^2) / C
    loss    = intra / (inter + 1e-6)

Per core (data-parallel over N):
  - one-hot(argmax) built on the vector engine (reduce_max + one
    broadcast is_equal over the whole supertile)
  - segment sums via PE: one matmul per 128-row group
    (one-hot^T @ features) accumulated in PSUM over the full loop
  - counts via PE with ones as the stationary operand
    (ones^T @ one-hot -> per-(j,c) column counts), two matmuls/supertile
  - sum-of-squares on the scalar engine (Square activation + accumulate)
Features stream as f32 -> bf16 cast DMAs (SWDGE), contiguous 8KB per
partition per supertile. The O(C*D) finalization runs on the host over
the 8 per-core partials (the gather/unshard step).
"""

import numpy as np

import concourse.bacc as bacc
import concourse.tile as tile
from concourse import mybir
from concourse.bass_utils import run_bass_kernel_spmd

N_CORES = 8
N_TOTAL = 262144
D = 256
C = 100
P = 128
T = 16  # 128-row groups per supertile (DMA batch)

F32 = mybir.dt.float32
BF16 = mybir.dt.bfloat16


def build_nc(rows_per_core: int, t: int = T, bufs: int = 6):
    """Build the per-core Bass program (same SPMD program on all cores)."""
    super_rows = P * t
    assert rows_per_core % super_rows == 0
    cc = 4  # j's per counts matmul (free dim cc*C <= 512)
    assert t % cc == 0
    n_super = rows_per_core // super_rows
    n_cnt = t // cc

    nc = bacc.Bacc(
        "TRN2", target_bir_lowering=False, debug=False, num_devices=N_CORES
    )

    feats = nc.dram_tensor(
        "features", [rows_per_core, D], F32, kind="ExternalInput"
    ).ap()
    labels = nc.dram_tensor(
        "labels", [rows_per_core, C], F32, kind="ExternalInput"
    ).ap()
    out_partial = nc.dram_tensor(
        "partial", [C, D], F32, kind="ExternalOutput"
    ).ap()
    out_counts = nc.dram_tensor(
        "counts", [1, t * C], F32, kind="ExternalOutput"
    ).ap()
    out_sqacc = nc.dram_tensor(
        "sqacc", [P, n_super], F32, kind="ExternalOutput"
    ).ap()

    # Blocked row mapping: row = s*(P*t) + p*t + j -> partition p reads t
    # contiguous rows per supertile (contiguous 8KB DRAM chunks per part).
    feats_v = feats.rearrange("(s p j) d -> s p j d", p=P, j=t)
    labels_v = labels.rearrange("(s p j) c -> s p j c", p=P, j=t)

    with tile.TileContext(nc) as tc:
        with (
            tc.tile_pool(name="feat", bufs=bufs) as feat_pool,
            tc.tile_pool(name="lbl", bufs=bufs) as lbl_pool,
            tc.tile_pool(name="oh", bufs=3) as oh_pool,
            tc.tile_pool(name="sq", bufs=2) as sq_pool,
            tc.tile_pool(name="acc", bufs=1) as acc_pool,
            tc.tile_pool(name="ps", bufs=1, space="PSUM") as psum_pool,
        ):
            psum_sums = psum_pool.tile([C, D], F32, tag="ps_sums")
            psum_cnt = [
                psum_pool.tile(
                    [1, cc * C], F32, tag=f"ps_cnt{k}", name=f"ps_cnt{k}"
                )
                for k in range(n_cnt)
            ]
            sqacc = acc_pool.tile([P, n_super], F32, tag="sqacc")
            ones = acc_pool.tile([P, 1], BF16, tag="ones")
            nc.vector.memset(ones[:, :], 1.0)

            for s in range(n_super):
                feat_t = feat_pool.tile([P, t, D], BF16)
                lbl_t = lbl_pool.tile([P, t, C], F32)
                # SWDGE (gpsimd) casts f32 -> bf16 during the transfer
                nc.gpsimd.dma_start(out=feat_t[:, :, :], in_=feats_v[s])
                nc.sync.dma_start(out=lbl_t[:, :, :], in_=labels_v[s])

                mx = oh_pool.tile([P, t], F32, tag="mx")
                onehot = oh_pool.tile([P, t, C], BF16, tag="oh")
                nc.vector.reduce_max(
                    mx[:, :], lbl_t[:, :, :], axis=mybir.AxisListType.X
                )
                mxb = mx[:, :].unsqueeze(-1).broadcast_to((P, t, C))
                nc.vector.tensor_tensor(
                    out=onehot[:, :, :],
                    in0=lbl_t[:, :, :],
                    in1=mxb,
                    op=mybir.AluOpType.is_equal,
                )

                sq_t = sq_pool.tile([P, t, D], BF16)
                nc.scalar.activation(
                    sq_t[:, :, :],
                    feat_t[:, :, :],
                    mybir.ActivationFunctionType.Square,
                    accum_out=sqacc[:, s : s + 1],
                )

                for j in range(t):
                    nc.tensor.matmul(
                        psum_sums[:, :],
                        onehot[:, j],
                        feat_t[:, j],
                        start=(s == 0 and j == 0),
                        stop=(s == n_super - 1 and j == t - 1),
                    )
                # counts: ones^T @ onehot -> column sums, per-(j,c)
                for k in range(n_cnt):
                    nc.tensor.matmul(
                        psum_cnt[k][:, :],
                        ones[:, :],
                        onehot[:, k * cc : (k + 1) * cc],
                        start=(s == 0),
                        stop=(s == n_super - 1),
                    )

            part_sb = acc_pool.tile([C, D], F32, tag="part")
            cnt_sb = acc_pool.tile([1, t * C], F32, tag="cnt")
            nc.vector.tensor_copy(part_sb[:, :], psum_sums[:, :])
            for k in range(n_cnt):
                nc.vector.tensor_copy(
                    cnt_sb[:, k * cc * C : (k + 1) * cc * C], psum_cnt[k][:, :]
                )
            nc.sync.dma_start(out=out_partial[:, :], in_=part_sb[:, :])
            nc.sync.dma_start(out=out_counts[:, :], in_=cnt_sb[:, :])
            nc.sync.dma_start(out=out_sqacc[:, :], in_=sqacc[:, :])

    nc.compile()
    return nc


_NC_CACHE: dict = {}


def _get_nc():
    if "nc" not in _NC_CACHE:
        _NC_CACHE["nc"] = build_nc(N_TOTAL // N_CORES)
    return _NC_CACHE["nc"]


def finalize(partials, countss, sqaccs):
    """Host gather/unshard: combine per-core partials into the scalar loss."""
    sums = np.zeros((C, D), np.float64)
    counts = np.zeros((C,), np.float64)
    sumsq = 0.0
    for part, cnt, sq in zip(partials, countss, sqaccs):
        sums += part.astype(np.float64)
        counts += cnt.astype(np.float64).reshape(-1, C).sum(axis=0)
        sumsq += float(sq.astype(np.float64).sum())
    centers = (
        np.where(counts[:, None] > 0, sums / np.maximum(counts, 1.0)[:, None], 0.0)
        + 1e-6
    )
    intra = (
        sumsq
        - 2.0 * float((sums * centers).sum())
        + float((counts * (centers**2).sum(axis=1)).sum())
    )
    cmean = centers.mean(axis=0, keepdims=True)
    inter = float(((centers - cmean) ** 2).sum()) / C
    loss = intra / (inter + 1e-6)
    return np.array(loss, dtype=np.float32)


def kernel(features: np.ndarray, labels: np.ndarray) -> np.ndarray:
    nc = _get_nc()
    rows = N_TOTAL // N_CORES
    in_maps = []
    for i in range(N_CORES):
        sl = slice(i * rows, (i + 1) * rows)
        in_maps.append(
            {
                "features": np.ascontiguousarray(features[sl], dtype=np.float32),
                "labels": np.ascontiguousarray(labels[sl], dtype=np.float32),
            }
        )
    res = run_bass_kernel_spmd(nc, in_maps, list(range(N_CORES)))
    return finalize(
        [r["partial"] for r in res.results],
        [r["counts"] for r in res.results],
        [r["sqacc"] for r in res.results],
    )
